# revision 13
# baseline (speedup 1.0000x reference)
"""Trainium2 Bass kernel for nn_Logalike_40072044871937 (v3).

Computes the Lorentz-hyperboloid CTMC log-likelihood:
    ll = sum_{c != i, s} log( pi * (P[c,s,0,si_s] * P[c,s,0,sj_cs]
                                    + [sj==si!=0] * P[c,s,si_s,si_s]^2) )
with P[c,s] = expm(t_c * Q_s),  t_c = 0.5 * arccosh(<x_i, x_c>_L clamp).

Rows of expm(t*Q) are Taylor series in dist = 2t.  With the positivity
shift B = Q + lam*I and host-staged row-power tables
R0[k, s, m] = (B_s^k)[0, m] / (k! 2^k), the device computes per core

    sigma0[c,s,m] = sum_k dist_c^k * R0[k,s,m]            (PE matmul)
    p0_sj[c,s]    = sigma0[c, s, char[c,s]]               (one-hot mult + grouped max)
    acc[c]        = sum_s ln(sig0si*p0_sj + same*sigssi^2)

and returns [128, 3] per-partition partials (acc, dist_j0, dist_j1); the
host applies the exp(-lam t) fold (-lam*S_sh*dist), the pi=1/n constant,
and subtracts the (masked) row-i contribution exactly.

Sharding: 8 cores = 2 cell-halves x 4 site-quarters -> per core 256 cells
(2 chunks of 128 partitions) x 64 sites.

v3 micro-design (v2 trace: XBAR DMA-transpose 1.2us each + serial, chains
interleaved on DVE doubling latency, grouped reduce runs 1x):
  - both ACT table sets (sqrt_and_others + natural_log) are co-resident;
    loads are forced to t=0 by two dummy activations and overlap the input
    DMA latency, so sqrt(u^2-1) is a single ACT op (bias folds the -1).
  - one merged arccosh chain on [128,2] (both cell-chunks as columns).
  - W^T via PE transpose (bf16 identity staged in the big blob) + DVE copy;
    per-chunk so chunk0's matmul starts before chunk1's transpose.
  - one-hot gather: DVE 2x multiply + grouped tensor_reduce(max).
  - no on-device final reduction: the [128,3] partials DMA out directly.
"""

import numpy as np
import ml_dtypes

import concourse.bacc as bacc
import concourse.tile as tile
import concourse.mybir as mybir
from concourse.bass_utils import run_bass_kernel_spmd

# problem shape (hardcoded per contract)
C, S, N, D = 512, 256, 16, 8
K = 16              # Taylor terms; ||dist*B/2||_inf <= ~1.7 -> term 15 < 1e-9
NCORES = 8
CH = 2              # cell chunks per core (128 cells each)
CPC = 256           # cells per core
SQ = 64             # sites per core
RHO = 1.0
UFIX = 3.0          # staged u-value for the masked row i (dist_i := acosh(3))
F32 = mybir.dt.float32
I32 = mybir.dt.int32
BF16 = mybir.dt.bfloat16
BF = ml_dtypes.bfloat16

_CACHE = {}

MAGIC = 0x5F3759E0  # 0x5f3759df + 1 (the +1 from the ~q identity)


def _build_nc():
    nc = bacc.Bacc("TRN2", target_bir_lowering=False, debug=False)
    AF = mybir.ActivationFunctionType
    ALU = mybir.AluOpType

    CW = SQ * N      # 1024 columns per cell-chunk
    SMB = CH * CW    # same-mask base column in big
    IDB = SMB + CH * SQ  # identity base column in big

    # misc bf16 [128, 20]: cols 0..17 Xa[p, j*9+d] = X[cell,d]*a9[d]
    # (row i -> [3,0..0]); 18..19 pad
    misc = nc.declare_dram_parameter("misc", [128, 20], BF16, isOutput=False)
    # big bf16 [128, 2304]: onehot[p, j*1024+s*16+m] | same[p, j*64+s] | ident
    big = nc.declare_dram_parameter("big", [128, IDB + 128], BF16,
                                    isOutput=False)
    # tab bf16 [K, 1152]: R0[k, s*16+m] | A0[k,s] | Ai[k,s]
    tab = nc.declare_dram_parameter("tab", [K, CW + 2 * SQ], BF16,
                                    isOutput=False)
    # out [128, 3]: col0 acc = sum_s ln(sigcombo); col1,2 dist per chunk
    out = nc.declare_dram_parameter("out", [128, 3], F32, isOutput=True)

    with tile.TileContext(nc) as tc:
        with (
            tc.tile_pool(name="consts", bufs=1) as consts,
            tc.tile_pool(name="work", bufs=1) as work,
            tc.tile_pool(name="pch", bufs=1, space="PSUM") as pch,
            tc.tile_pool(name="paux", bufs=1, space="PSUM") as paux,
        ):
            # ---------- t0: no-dependency ops ----------
            s_dummy = work.tile([1, 1], F32)
            nc.gpsimd.memset(s_dummy[:], 1.0)
            # force the single ACT table set (natural_log) to load at t=0
            s_dmyo = work.tile([1, 1], F32)
            nc.scalar.activation(s_dmyo[:], s_dummy[:], AF.Ln)
            # W powers tile [128, (j,k)] bf16; k=0 columns preset to 1
            s_w = work.tile([128, CH * K], BF16)
            nc.vector.memset(s_w[:, 0:1], 1.0)
            nc.vector.memset(s_w[:, K:K + 1], 1.0)

            # ---------- input DMAs (all SP-triggered HWDGE) ----------
            s_misc = consts.tile([128, 20], BF16)
            nc.sync.dma_start(s_misc[:], misc[:])
            s_big = consts.tile([128, IDB + 128], BF16)
            nc.sync.dma_start(s_big[:], big[:])
            s_tab = consts.tile([K, CW + 2 * SQ], BF16)
            nc.sync.dma_start(s_tab[:], tab[:])

            # ---------- arccosh chain on [128, 2] ----------
            s_u = work.tile([128, CH], F32)
            nc.vector.tensor_reduce(
                out=s_u[:], in_=s_misc[:, 0:CH * 9].rearrange(
                    "p (j d) -> p j d", d=9),
                axis=mybir.AxisListType.X, op=ALU.add)
            s_sq = work.tile([128, CH], F32)
            nc.vector.tensor_mul(s_sq[:], s_u[:], s_u[:])
            s_y = work.tile([128, CH], F32)
            nc.vector.tensor_scalar_add(s_y[:], s_sq[:], -1.0)
            # fast inverse sqrt + one Newton step (avoids the Sqrt ACT
            # table: only natural_log is ever resident -> zero reloads)
            s_qi = work.tile([128, CH], I32)
            nc.vector.tensor_scalar(out=s_qi[:], in0=s_y[:].bitcast(I32),
                                    scalar1=1, scalar2=-1,
                                    op0=ALU.logical_shift_right,
                                    op1=ALU.bitwise_xor)
            s_zi = work.tile([128, CH], I32)
            nc.vector.tensor_scalar_add(s_zi[:], s_qi[:], MAGIC)
            z0 = s_zi[:].bitcast(F32)
            s_t1 = work.tile([128, CH], F32)
            nc.vector.tensor_mul(s_t1[:], z0, z0)
            s_t2 = work.tile([128, CH], F32)
            nc.vector.tensor_mul(s_t2[:], s_t1[:], s_y[:])
            s_h = work.tile([128, CH], F32)
            nc.vector.tensor_scalar(out=s_h[:], in0=s_t2[:], scalar1=-0.5,
                                    scalar2=1.5, op0=ALU.mult, op1=ALU.add)
            s_z1 = work.tile([128, CH], F32)
            nc.vector.tensor_mul(s_z1[:], z0, s_h[:])
            s_m = work.tile([128, CH], F32)
            nc.vector.tensor_mul(s_m[:], s_y[:], s_z1[:])
            s_v = work.tile([128, CH], F32)
            nc.vector.tensor_add(s_v[:], s_m[:], s_u[:])
            s_d = work.tile([128, CH], F32)
            nc.scalar.activation(s_d[:], s_v[:], AF.Ln)

            # ---------- W powers: s_w[:, j*K+k] = dist_j^k (bf16) ----------
            wv = s_w[:].rearrange("p (j k) -> p j k", j=CH)
            nc.vector.tensor_copy(s_w[:, 1:CH * K:K], s_d[:])  # k=1 columns
            e2 = work.tile([128, CH], F32)
            nc.vector.tensor_mul(e2[:], s_d[:], s_d[:])
            nc.vector.tensor_mul(wv[:, :, 2:4], wv[:, :, 0:2],
                                 e2[:].rearrange("p (j o) -> p j o", o=1)
                                 .broadcast_to([128, CH, 2]))
            e4 = work.tile([128, CH], F32)
            nc.vector.tensor_mul(e4[:], e2[:], e2[:])
            nc.vector.tensor_mul(wv[:, :, 4:8], wv[:, :, 0:4],
                                 e4[:].rearrange("p (j o) -> p j o", o=1)
                                 .broadcast_to([128, CH, 4]))
            e8 = work.tile([128, CH], F32)
            nc.vector.tensor_mul(e8[:], e4[:], e4[:])
            nc.vector.tensor_mul(wv[:, :, 8:16], wv[:, :, 0:8],
                                 e8[:].rearrange("p (j o) -> p j o", o=1)
                                 .broadcast_to([128, CH, 8]))

            # ---------- W^T per chunk: PE transpose + DVE copy ----------
            s_wts = []
            for j in range(CH):
                p_wt = paux.tile([K, 128], BF16, name=f"pwt{j}")
                nc.tensor.transpose(p_wt[:], s_w[:, j * K:(j + 1) * K],
                                    s_big[:, IDB:IDB + 128])
                s_wt = work.tile([K, 128], BF16, name=f"swt{j}")
                nc.vector.tensor_copy(s_wt[:], p_wt[:])
                s_wts.append(s_wt)

            # ---------- PE: sigma matmuls ----------
            p_ch0 = pch.tile([128, CW], F32)
            p_ch1 = pch.tile([128, CW], F32)
            p_sig = paux.tile([128, CH * 2 * SQ], F32)
            for j, p_ch in enumerate((p_ch0, p_ch1)):
                lhsT = s_wts[j][:]
                for h in range(CW // 512):
                    nc.tensor.matmul(
                        p_ch[:, h * 512:(h + 1) * 512], lhsT,
                        s_tab[:, h * 512:(h + 1) * 512],
                        start=True, stop=True)
                nc.tensor.matmul(p_sig[:, j * 2 * SQ:(j + 1) * 2 * SQ], lhsT,
                                 s_tab[:, CW:CW + 2 * SQ],
                                 start=True, stop=True)

            # ---------- per-chunk: copy -> one-hot mult -> grouped max ----
            s_p0sj = work.tile([128, CH * SQ], BF16)
            for j, p_ch in enumerate((p_ch0, p_ch1)):
                s_p0b = work.tile([128, CW], BF16, name=f"p0b{j}")
                nc.scalar.copy(s_p0b[:], p_ch[:])
                s_m = work.tile([128, CW], BF16, name=f"mm{j}")
                nc.vector.tensor_mul(s_m[:], s_p0b[:],
                                     s_big[:, j * CW:(j + 1) * CW])
                nc.vector.tensor_reduce(
                    out=s_p0sj[:, j * SQ:(j + 1) * SQ],
                    in_=s_m[:].rearrange("p (s m) -> p s m", m=N),
                    axis=mybir.AxisListType.X, op=ALU.max)

            # ---------- combine ----------
            sigv = p_sig[:].rearrange("p (j t) -> p j t", j=CH)
            s_p0t = work.tile([128, CH * SQ], BF16)
            nc.vector.tensor_mul(
                s_p0t[:].rearrange("p (j s) -> p j s", j=CH),
                s_p0sj[:].rearrange("p (j s) -> p j s", j=CH),
                sigv[:, :, 0:SQ])
            s_ssm = work.tile([128, CH * SQ], BF16)
            nc.vector.tensor_mul(
                s_ssm[:].rearrange("p (j s) -> p j s", j=CH),
                sigv[:, :, SQ:2 * SQ],
                s_big[:, SMB:SMB + CH * SQ].rearrange(
                    "p (j s) -> p j s", j=CH))
            s_ss2 = work.tile([128, CH * SQ], BF16)
            nc.vector.tensor_mul(s_ss2[:], s_ssm[:], s_ssm[:])
            s_cur = work.tile([128, CH * SQ], F32)
            nc.vector.tensor_add(s_cur[:], s_p0t[:], s_ss2[:])

            # ---------- ln + accum; ship [acc | dist] ----------
            s_res = work.tile([128, 3], F32)
            nc.vector.tensor_copy(s_res[:, 1:3], s_d[:])
            s_lnout = work.tile([128, CH * SQ], F32)
            nc.scalar.activation(s_lnout[:], s_cur[:], AF.Ln)
            nc.vector.tensor_reduce(out=s_res[:, 0:1], in_=s_lnout[:],
                                    axis=mybir.AxisListType.X, op=ALU.add)
            nc.sync.dma_start(out[:], s_res[:])

    nc.finalize()
    return nc


def _host_prep(X, Q, char, i):
    """Shard + stage tables (O(S*K*n^2) host work, same class as v1)."""
    X = np.asarray(X, np.float32)
    Q = np.asarray(Q, np.float32)
    char = np.asarray(char, np.int32)
    i = int(np.asarray(i))
    has_i = 0 <= i < C

    lam = float(np.max(-np.diagonal(Q, axis1=-2, axis2=-1)).astype(np.float64))
    Bd = Q.astype(np.float64) + lam * np.eye(N)
    si = char[i] if has_i else np.zeros(S, np.int32)  # [S]

    # row-power tables with 1/(k! 2^k) folded in (t = dist/2)
    R0 = np.zeros((K, S, N), np.float64)
    Ri_ss = np.zeros((K, S), np.float64)
    r0 = np.zeros((S, N)); r0[:, 0] = 1.0
    ri = np.zeros((S, N)); ri[np.arange(S), si] = 1.0
    scale = 1.0
    for k in range(K):
        if k > 0:
            scale *= 2.0 * k
            r0 = np.einsum('sp,spm->sm', r0, Bd)
            ri = np.einsum('sp,spm->sm', ri, Bd)
        R0[k] = r0 / scale
        Ri_ss[k] = ri[np.arange(S), si] / scale
    A0 = R0[:, np.arange(S), si]
    Ai = Ri_ss.copy()
    Ai[:, si == 0] = 0.0
    # bf16-rounded copies (match what the device computes with)
    R0b = R0.astype(BF).astype(np.float64)
    A0b = A0.astype(BF).astype(np.float64)
    Aib = Ai.astype(BF).astype(np.float64)

    xi = X[i] if has_i else X[0]
    a9 = np.empty(D + 1, np.float64)
    a9[0] = xi[0] / RHO
    a9[1:] = -xi[1:].astype(np.float64) / RHO
    Xa = X.astype(np.float64) * a9[None, :]          # [C, 9]
    if has_i:
        Xa[i, :] = 0.0
        Xa[i, 0] = UFIX

    oh = (char[:, :, None] == np.arange(N)[None, None, :])  # [C, S, N]
    ident = np.eye(128, dtype=np.float64)

    in_maps = []
    for core in range(NCORES):
        h, q = core // 4, core % 4
        cells = h * CPC + np.arange(CPC)                 # [256]
        g = cells.reshape(CH, 128)                       # [j, p]
        ts = slice(q * SQ, (q + 1) * SQ)
        sisl = si[ts]

        misc = np.zeros((128, 20), np.float64)
        misc[:, 0:18] = Xa[g].transpose(1, 0, 2).reshape(128, 18)

        ohc = oh[g][:, :, ts, :]                         # [j, p, s, m]
        bigm = np.empty((128, CH * SQ * N + CH * SQ + 128), np.float64)
        bigm[:, 0:CH * SQ * N] = ohc.transpose(1, 0, 2, 3).reshape(128, -1)
        same = ((char[g][:, :, ts] == sisl[None, None, :])
                & (sisl[None, None, :] != 0))            # [j, p, s]
        bigm[:, CH * SQ * N:CH * SQ * N + CH * SQ] = (
            same.transpose(1, 0, 2).reshape(128, -1))
        bigm[:, CH * SQ * N + CH * SQ:] = ident

        tabm = np.empty((K, SQ * N + 2 * SQ), np.float64)
        tabm[:, 0:SQ * N] = R0[:, ts, :].reshape(K, -1)
        tabm[:, SQ * N:SQ * N + SQ] = A0[:, ts]
        tabm[:, SQ * N + SQ:] = Ai[:, ts]

        in_maps.append({
            "misc": np.ascontiguousarray(misc.astype(BF)),
            "big": np.ascontiguousarray(bigm.astype(BF)),
            "tab": np.ascontiguousarray(tabm.astype(BF)),
        })

    n_valid = C - (1 if has_i else 0)
    host_const = float(n_valid) * float(S) * float(np.log(1.0 / N))
    if has_i:
        # row i is staged with u=3 (dist=acosh(3)); remove its device
        # contribution: the -lam*SQ*dist fold (host applies it for all
        # cells) and its ln-sum (recomputed here with the bf16 tables).
        dfix = float(np.arccosh(UFIX))
        host_const += S * lam * dfix
        pw = dfix ** np.arange(K)                        # [K]
        sig0si = pw @ A0b                                # [S]
        sigssi = pw @ Aib                                # [S]
        cur_i = sig0si * sig0si + (si != 0) * sigssi * sigssi
        host_const -= float(np.sum(np.log(cur_i)))
    return host_const, lam, in_maps


def run(X, Q, char, i, trace=False):
    if "nc" not in _CACHE:
        _CACHE["nc"] = _build_nc()
    nc = _CACHE["nc"]
    host_const, lam, in_maps = _host_prep(X, Q, char, i)
    res = run_bass_kernel_spmd(nc, in_maps, core_ids=list(range(NCORES)),
                               trace=trace)
    total = host_const
    for r in res.results:
        o = np.asarray(r["out"], np.float64)
        total += float(np.sum(o[:, 0])) - SQ * lam * float(
            np.sum(o[:, 1]) + np.sum(o[:, 2]))
    return np.asarray(total, dtype=np.float32), res


def kernel(X, Q, char, i):
    out, _ = run(X, Q, char, i)
    return out


# revision 14
# speedup vs baseline: 1.0835x; 1.0835x over previous
"""Trainium2 Bass kernel for nn_Logalike_40072044871937 (v3).

Computes the Lorentz-hyperboloid CTMC log-likelihood:
    ll = sum_{c != i, s} log( pi * (P[c,s,0,si_s] * P[c,s,0,sj_cs]
                                    + [sj==si!=0] * P[c,s,si_s,si_s]^2) )
with P[c,s] = expm(t_c * Q_s),  t_c = 0.5 * arccosh(<x_i, x_c>_L clamp).

Rows of expm(t*Q) are Taylor series in dist = 2t.  With the positivity
shift B = Q + lam*I and host-staged row-power tables
R0[k, s, m] = (B_s^k)[0, m] / (k! 2^k), the device computes per core

    sigma0[c,s,m] = sum_k dist_c^k * R0[k,s,m]            (PE matmul)
    p0_sj[c,s]    = sigma0[c, s, char[c,s]]               (one-hot mult + grouped max)
    acc[c]        = sum_s ln(sig0si*p0_sj + same*sigssi^2)

and returns [128, 3] per-partition partials (acc, dist_j0, dist_j1); the
host applies the exp(-lam t) fold (-lam*S_sh*dist), the pi=1/n constant,
and subtracts the (masked) row-i contribution exactly.

Sharding: 8 cores = 2 cell-halves x 4 site-quarters -> per core 256 cells
(2 chunks of 128 partitions) x 64 sites.

v3 micro-design (v2 trace: XBAR DMA-transpose 1.2us each + serial, chains
interleaved on DVE doubling latency, grouped reduce runs 1x):
  - both ACT table sets (sqrt_and_others + natural_log) are co-resident;
    loads are forced to t=0 by two dummy activations and overlap the input
    DMA latency, so sqrt(u^2-1) is a single ACT op (bias folds the -1).
  - one merged arccosh chain on [128,2] (both cell-chunks as columns).
  - W^T via PE transpose (bf16 identity staged in the big blob) + DVE copy;
    per-chunk so chunk0's matmul starts before chunk1's transpose.
  - one-hot gather: DVE 2x multiply + grouped tensor_reduce(max).
  - no on-device final reduction: the [128,3] partials DMA out directly.
"""

import numpy as np
import ml_dtypes

import concourse.bacc as bacc
import concourse.tile as tile
import concourse.mybir as mybir
from concourse.bass_utils import run_bass_kernel_spmd

# problem shape (hardcoded per contract)
C, S, N, D = 512, 256, 16, 8
K = 8               # Taylor terms; ||dist*B/2||_inf <= ~1.7 -> ll bias ~3e-5
NCORES = 8
CH = 2              # cell chunks per core (128 cells each)
CPC = 256           # cells per core
SQ = 64             # sites per core
RHO = 1.0
UFIX = 3.0          # staged u-value for the masked row i (dist_i := acosh(3))
F32 = mybir.dt.float32
I32 = mybir.dt.int32
BF16 = mybir.dt.bfloat16
BF = ml_dtypes.bfloat16

_CACHE = {}

MAGIC = 0x5F3759E0  # 0x5f3759df + 1 (the +1 from the ~q identity)


def _build_nc():
    nc = bacc.Bacc("TRN2", target_bir_lowering=False, debug=False)
    AF = mybir.ActivationFunctionType
    ALU = mybir.AluOpType

    CW = SQ * N      # 1024 columns per cell-chunk
    SMB = CH * CW    # same-mask base column in big
    IDB = SMB + CH * SQ  # identity base column in big

    # misc bf16 [128, 20]: cols 0..17 Xa[p, j*9+d] = X[cell,d]*a9[d]
    # (row i -> [3,0..0]); 18..19 pad
    misc = nc.declare_dram_parameter("misc", [128, 20], BF16, isOutput=False)
    # big bf16 [128, 2304]: onehot[p, j*1024+s*16+m] | same[p, j*64+s] | ident
    big = nc.declare_dram_parameter("big", [128, IDB + 128], BF16,
                                    isOutput=False)
    # tab bf16 [K, 1152]: R0[k, s*16+m] | A0[k,s] | Ai[k,s]
    tab = nc.declare_dram_parameter("tab", [K, CW + 2 * SQ], BF16,
                                    isOutput=False)
    # out [128, 3]: col0 acc = sum_s ln(sigcombo); col1,2 dist per chunk
    out = nc.declare_dram_parameter("out", [128, 3], F32, isOutput=True)

    with tile.TileContext(nc) as tc:
        with (
            tc.tile_pool(name="consts", bufs=1) as consts,
            tc.tile_pool(name="work", bufs=1) as work,
            tc.tile_pool(name="pch", bufs=1, space="PSUM") as pch,
            tc.tile_pool(name="paux", bufs=1, space="PSUM") as paux,
        ):
            # ---------- t0: no-dependency ops ----------
            s_dummy = work.tile([1, 1], F32)
            nc.gpsimd.memset(s_dummy[:], 1.0)
            # force the single ACT table set (natural_log) to load at t=0
            s_dmyo = work.tile([1, 1], F32)
            nc.scalar.activation(s_dmyo[:], s_dummy[:], AF.Ln)
            # W powers tile [128, (j,k)] bf16; k=0 columns preset to 1
            s_w = work.tile([128, CH * K], BF16)
            nc.vector.memset(s_w[:, 0:1], 1.0)
            nc.vector.memset(s_w[:, K:K + 1], 1.0)

            # ---------- input DMAs (all SP-triggered HWDGE) ----------
            s_misc = consts.tile([128, 20], BF16)
            nc.sync.dma_start(s_misc[:], misc[:])
            s_big = consts.tile([128, IDB + 128], BF16)
            nc.sync.dma_start(s_big[:], big[:])
            s_tab = consts.tile([K, CW + 2 * SQ], BF16)
            nc.sync.dma_start(s_tab[:], tab[:])

            # ---------- arccosh chain on [128, 2] ----------
            s_u = work.tile([128, CH], F32)
            nc.vector.tensor_reduce(
                out=s_u[:], in_=s_misc[:, 0:CH * 9].rearrange(
                    "p (j d) -> p j d", d=9),
                axis=mybir.AxisListType.X, op=ALU.add)
            s_sq = work.tile([128, CH], F32)
            nc.vector.tensor_mul(s_sq[:], s_u[:], s_u[:])
            s_y = work.tile([128, CH], F32)
            nc.vector.tensor_scalar_add(s_y[:], s_sq[:], -1.0)
            # fast inverse sqrt + one Newton step (avoids the Sqrt ACT
            # table: only natural_log is ever resident -> zero reloads)
            s_qi = work.tile([128, CH], I32)
            nc.vector.tensor_scalar(out=s_qi[:], in0=s_y[:].bitcast(I32),
                                    scalar1=1, scalar2=-1,
                                    op0=ALU.logical_shift_right,
                                    op1=ALU.bitwise_xor)
            s_zi = work.tile([128, CH], I32)
            nc.vector.tensor_scalar_add(s_zi[:], s_qi[:], MAGIC)
            z0 = s_zi[:].bitcast(F32)
            s_t1 = work.tile([128, CH], F32)
            nc.vector.tensor_mul(s_t1[:], z0, z0)
            s_t2 = work.tile([128, CH], F32)
            nc.vector.tensor_mul(s_t2[:], s_t1[:], s_y[:])
            s_h = work.tile([128, CH], F32)
            nc.vector.tensor_scalar(out=s_h[:], in0=s_t2[:], scalar1=-0.5,
                                    scalar2=1.5, op0=ALU.mult, op1=ALU.add)
            s_z1 = work.tile([128, CH], F32)
            nc.vector.tensor_mul(s_z1[:], z0, s_h[:])
            s_m = work.tile([128, CH], F32)
            nc.vector.tensor_mul(s_m[:], s_y[:], s_z1[:])
            s_v = work.tile([128, CH], F32)
            nc.vector.tensor_add(s_v[:], s_m[:], s_u[:])
            s_d = work.tile([128, CH], F32)
            nc.scalar.activation(s_d[:], s_v[:], AF.Ln)

            # ---------- W powers per chunk (chunk0 first so mm0 starts
            # early): s_w[:, j*K+k] = dist_j^k (bf16), then PE transpose ----
            s_wts = []
            for j in range(CH):
                wj = s_w[:, j * K:(j + 1) * K]
                dj = s_d[:, j:j + 1]
                nc.vector.tensor_copy(wj[:, 1:2], dj)
                e2 = work.tile([128, 1], F32, name=f"e2{j}")
                nc.vector.tensor_mul(e2[:], dj, dj)
                nc.vector.tensor_mul(wj[:, 2:4], wj[:, 0:2],
                                     e2[:].broadcast_to([128, 2]))
                e4 = work.tile([128, 1], F32, name=f"e4{j}")
                nc.vector.tensor_mul(e4[:], e2[:], e2[:])
                nc.vector.tensor_mul(wj[:, 4:8], wj[:, 0:4],
                                     e4[:].broadcast_to([128, 4]))
                p_wt = paux.tile([K, 128], BF16, name=f"pwt{j}")
                nc.tensor.transpose(p_wt[:], wj, s_big[:, IDB:IDB + 128])
                s_wt = work.tile([K, 128], BF16, name=f"swt{j}")
                nc.vector.tensor_copy(s_wt[:], p_wt[:])
                s_wts.append(s_wt)

            # ---------- PE: sigma matmuls ----------
            p_ch0 = pch.tile([128, CW], F32)
            p_ch1 = pch.tile([128, CW], F32)
            p_sig = paux.tile([128, CH * 2 * SQ], F32)
            for j, p_ch in enumerate((p_ch0, p_ch1)):
                lhsT = s_wts[j][:]
                for h in range(CW // 512):
                    nc.tensor.matmul(
                        p_ch[:, h * 512:(h + 1) * 512], lhsT,
                        s_tab[:, h * 512:(h + 1) * 512],
                        start=True, stop=True)
                nc.tensor.matmul(p_sig[:, j * 2 * SQ:(j + 1) * 2 * SQ], lhsT,
                                 s_tab[:, CW:CW + 2 * SQ],
                                 start=True, stop=True)

            # ---------- per-chunk: copy -> one-hot mult -> grouped max ----
            s_p0sj = work.tile([128, CH * SQ], BF16)
            for j, p_ch in enumerate((p_ch0, p_ch1)):
                s_p0b = work.tile([128, CW], BF16, name=f"p0b{j}")
                nc.scalar.copy(s_p0b[:, 0:512], p_ch[:, 0:512])
                nc.vector.tensor_copy(s_p0b[:, 512:CW], p_ch[:, 512:CW])
                s_m = work.tile([128, CW], BF16, name=f"mm{j}")
                nc.vector.tensor_mul(s_m[:], s_p0b[:],
                                     s_big[:, j * CW:(j + 1) * CW])
                # one 2x-mode pairwise-max level, then the grouped reduce
                mv = s_m[:].rearrange("p (s m) -> p s m", m=N)
                s_l = work.tile([128, CW // 2], BF16, name=f"l1{j}")
                nc.vector.tensor_tensor(
                    out=s_l[:].rearrange("p (s m) -> p s m", m=N // 2),
                    in0=mv[:, :, 0:N // 2], in1=mv[:, :, N // 2:N],
                    op=ALU.max)
                nc.vector.tensor_reduce(
                    out=s_p0sj[:, j * SQ:(j + 1) * SQ],
                    in_=s_l[:].rearrange("p (s m) -> p s m", m=N // 2),
                    axis=mybir.AxisListType.X, op=ALU.max)

            # ---------- combine ----------
            sigv = p_sig[:].rearrange("p (j t) -> p j t", j=CH)
            s_p0t = work.tile([128, CH * SQ], BF16)
            nc.vector.tensor_mul(
                s_p0t[:].rearrange("p (j s) -> p j s", j=CH),
                s_p0sj[:].rearrange("p (j s) -> p j s", j=CH),
                sigv[:, :, 0:SQ])
            s_ssm = work.tile([128, CH * SQ], BF16)
            nc.vector.tensor_mul(
                s_ssm[:].rearrange("p (j s) -> p j s", j=CH),
                sigv[:, :, SQ:2 * SQ],
                s_big[:, SMB:SMB + CH * SQ].rearrange(
                    "p (j s) -> p j s", j=CH))
            s_ss2 = work.tile([128, CH * SQ], BF16)
            nc.vector.tensor_mul(s_ss2[:], s_ssm[:], s_ssm[:])
            s_cur = work.tile([128, CH * SQ], F32)
            nc.vector.tensor_add(s_cur[:], s_p0t[:], s_ss2[:])

            # ---------- ln + accum; ship [acc | dist] ----------
            s_res = work.tile([128, 3], F32)
            nc.vector.tensor_copy(s_res[:, 1:3], s_d[:])
            s_lnout = work.tile([128, CH * SQ], BF16)
            nc.scalar.activation(s_lnout[:], s_cur[:], AF.Ln,
                                 accum_out=s_res[:, 0:1])
            nc.sync.dma_start(out[:], s_res[:])

    nc.finalize()
    return nc


def _host_prep(X, Q, char, i):
    """Shard + stage tables (O(S*K*n^2) host work, same class as v1)."""
    X = np.asarray(X, np.float32)
    Q = np.asarray(Q, np.float32)
    char = np.asarray(char, np.int32)
    i = int(np.asarray(i))
    has_i = 0 <= i < C

    lam = float(np.max(-np.diagonal(Q, axis1=-2, axis2=-1)).astype(np.float64))
    Bd = Q.astype(np.float64) + lam * np.eye(N)
    si = char[i] if has_i else np.zeros(S, np.int32)  # [S]

    # row-power tables with 1/(k! 2^k) folded in (t = dist/2)
    R0 = np.zeros((K, S, N), np.float64)
    Ri_ss = np.zeros((K, S), np.float64)
    r0 = np.zeros((S, N)); r0[:, 0] = 1.0
    ri = np.zeros((S, N)); ri[np.arange(S), si] = 1.0
    scale = 1.0
    for k in range(K):
        if k > 0:
            scale *= 2.0 * k
            r0 = np.einsum('sp,spm->sm', r0, Bd)
            ri = np.einsum('sp,spm->sm', ri, Bd)
        R0[k] = r0 / scale
        Ri_ss[k] = ri[np.arange(S), si] / scale
    A0 = R0[:, np.arange(S), si]
    Ai = Ri_ss.copy()
    Ai[:, si == 0] = 0.0
    # bf16-rounded copies (match what the device computes with)
    R0b = R0.astype(BF).astype(np.float64)
    A0b = A0.astype(BF).astype(np.float64)
    Aib = Ai.astype(BF).astype(np.float64)

    xi = X[i] if has_i else X[0]
    a9 = np.empty(D + 1, np.float64)
    a9[0] = xi[0] / RHO
    a9[1:] = -xi[1:].astype(np.float64) / RHO
    Xa = X.astype(np.float64) * a9[None, :]          # [C, 9]
    if has_i:
        Xa[i, :] = 0.0
        Xa[i, 0] = UFIX

    oh = (char[:, :, None] == np.arange(N)[None, None, :])  # [C, S, N]
    ident = np.eye(128, dtype=np.float64)

    in_maps = []
    for core in range(NCORES):
        h, q = core // 4, core % 4
        cells = h * CPC + np.arange(CPC)                 # [256]
        g = cells.reshape(CH, 128)                       # [j, p]
        ts = slice(q * SQ, (q + 1) * SQ)
        sisl = si[ts]

        misc = np.zeros((128, 20), np.float64)
        misc[:, 0:18] = Xa[g].transpose(1, 0, 2).reshape(128, 18)

        ohc = oh[g][:, :, ts, :]                         # [j, p, s, m]
        bigm = np.empty((128, CH * SQ * N + CH * SQ + 128), np.float64)
        bigm[:, 0:CH * SQ * N] = ohc.transpose(1, 0, 2, 3).reshape(128, -1)
        same = ((char[g][:, :, ts] == sisl[None, None, :])
                & (sisl[None, None, :] != 0))            # [j, p, s]
        bigm[:, CH * SQ * N:CH * SQ * N + CH * SQ] = (
            same.transpose(1, 0, 2).reshape(128, -1))
        bigm[:, CH * SQ * N + CH * SQ:] = ident

        tabm = np.empty((K, SQ * N + 2 * SQ), np.float64)
        tabm[:, 0:SQ * N] = R0[:, ts, :].reshape(K, -1)
        tabm[:, SQ * N:SQ * N + SQ] = A0[:, ts]
        tabm[:, SQ * N + SQ:] = Ai[:, ts]

        in_maps.append({
            "misc": np.ascontiguousarray(misc.astype(BF)),
            "big": np.ascontiguousarray(bigm.astype(BF)),
            "tab": np.ascontiguousarray(tabm.astype(BF)),
        })

    n_valid = C - (1 if has_i else 0)
    host_const = float(n_valid) * float(S) * float(np.log(1.0 / N))
    if has_i:
        # row i is staged with u=3 (dist=acosh(3)); remove its device
        # contribution: the -lam*SQ*dist fold (host applies it for all
        # cells) and its ln-sum (recomputed here with the bf16 tables).
        dfix = float(np.arccosh(UFIX))
        host_const += S * lam * dfix
        pw = dfix ** np.arange(K)                        # [K]
        sig0si = pw @ A0b                                # [S]
        sigssi = pw @ Aib                                # [S]
        cur_i = sig0si * sig0si + (si != 0) * sigssi * sigssi
        host_const -= float(np.sum(np.log(cur_i)))
    return host_const, lam, in_maps


def run(X, Q, char, i, trace=False):
    if "nc" not in _CACHE:
        _CACHE["nc"] = _build_nc()
    nc = _CACHE["nc"]
    host_const, lam, in_maps = _host_prep(X, Q, char, i)
    res = run_bass_kernel_spmd(nc, in_maps, core_ids=list(range(NCORES)),
                               trace=trace)
    total = host_const
    for r in res.results:
        o = np.asarray(r["out"], np.float64)
        total += float(np.sum(o[:, 0])) - SQ * lam * float(
            np.sum(o[:, 1]) + np.sum(o[:, 2]))
    return np.asarray(total, dtype=np.float32), res


def kernel(X, Q, char, i):
    out, _ = run(X, Q, char, i)
    return out
